# revision 1
# baseline (speedup 1.0000x reference)
"""AttentionBlock (GroupNorm + 8-head self-attention + proj + residual) on 8 trn2 cores.

Sharding: data-parallel over batch B=16 -> 2 samples per core. No collectives.

Per-sample dataflow (C=512 channels, L=1024 pixels, 8 heads x 64 dims):
  - x (C, L) lives as 4 SBUF tiles (128, 1024), channels on partitions.
  - GroupNorm: per-channel mean/var via bn_stats over L; 16-channel group
    aggregation + broadcast-back via tiny mask matmuls on the PE.
  - QKV: q^T,k^T computed as (channels, L) tiles; v computed directly in
    (L, channels) orientation (lhsT = h) so AV needs no transposes.
  - Attention per head pair, split by i-halves so PSUM double-buffers:
    S^T = k^T.T @ q^T chunks (row-packed head pairs share the PE array, K=64
    each); exp on ScalarE with the 1/8 scale fused, PSUM (128,1024) in one
    instruction; AV uses v' = [v | ones] (M=65) so the softmax denominator
    rides along as PSUM row 64. Denominators collect per pair into a (2, L)
    tile; reciprocal + a K=2 selector matmul broadcasts them back to channel
    rows for one normalization multiply per chunk, right after each pair.
  - proj + bias + residual, write out.
  - Cross-sample software pipeline: sample s+1's groupnorm/QKV/V fill the PE
    while ScalarE works through sample s's exps; sample s's proj fills the
    head of sample s+1's attention.

Big matmuls run in float32r (full-rate PE mode, ~1e-4 rel err); producers of
their operands write float32r-rounded outputs as walrus requires.
"""

import numpy as np

import concourse.bass as bass
import concourse.mybir as mybir
import concourse.tile as tile
from concourse import bacc
from concourse.bass_utils import run_bass_kernel_spmd
from concourse.masks import make_identity

F32 = mybir.dt.float32
F32R = mybir.dt.float32r
AF = mybir.ActivationFunctionType
OP = mybir.AluOpType

B, C, H, W = 16, 512, 32, 32
L = H * W
NH, HD = 8, 64
NG, GS = 32, 16
EPS = 1e-5
N_CORES = 8
BPC = B // N_CORES  # samples per core
P = 128
CK = C // P   # 4 channel chunks
LK = L // P   # 8 pixel chunks
SCALE = HD ** -0.5

_NC_CACHE = {}


class Ctx:
    pass


def _consts(nc, const, nw_d, nb_d, qb_d, pb_d):
    c = Ctx()
    c.ident = const.tile([P, P], F32, tag="ident")
    make_identity(nc, c.ident)

    # gmask[kc][ch, g] = 1/16 iff global_channel // 16 == g   (128, 32)
    c.gmask = []
    for kc in range(CK):
        gm = const.tile([P, NG], F32, tag=f"gmask{kc}", name=f"gmask{kc}")
        nc.gpsimd.memset(gm, 1.0 / GS)
        nc.gpsimd.affine_select(
            out=gm, in_=gm, compare_op=OP.is_ge, fill=0.0,
            base=P * kc, channel_multiplier=1, pattern=[[-GS, NG]])
        nc.gpsimd.affine_select(
            out=gm, in_=gm, compare_op=OP.is_ge, fill=0.0,
            base=(GS - 1) - P * kc, channel_multiplier=-1, pattern=[[GS, NG]])
        c.gmask.append(gm)

    # sel2[h2, ch] = 1 iff ch // 64 == h2  (2, 128), f32r for full-rate matmul
    sel2s = const.tile([2, P], F32, tag="sel2s")
    nc.gpsimd.memset(sel2s, 1.0)
    nc.gpsimd.affine_select(
        out=sel2s, in_=sel2s, compare_op=OP.is_ge, fill=0.0,
        base=0, channel_multiplier=-HD, pattern=[[1, P]])
    nc.gpsimd.affine_select(
        out=sel2s, in_=sel2s, compare_op=OP.is_ge, fill=0.0,
        base=HD - 1, channel_multiplier=HD, pattern=[[-1, P]])
    c.sel2 = const.tile([2, P], F32R, tag="sel2")
    nc.vector.tensor_copy(out=c.sel2, in_=sel2s)

    # bmask[g, ch] = 1 iff ch // 16 == g  (32, 512)
    c.bmask = const.tile([NG, C], F32, tag="bmask")
    nc.gpsimd.memset(c.bmask, 1.0)
    nc.gpsimd.affine_select(
        out=c.bmask, in_=c.bmask, compare_op=OP.is_ge, fill=0.0,
        base=0, channel_multiplier=-GS, pattern=[[1, C]])
    nc.gpsimd.affine_select(
        out=c.bmask, in_=c.bmask, compare_op=OP.is_ge, fill=0.0,
        base=GS - 1, channel_multiplier=GS, pattern=[[-1, C]])

    nw_r = nw_d.ap().rearrange("(kc p) -> kc p", p=P)
    nb_r = nb_d.ap().rearrange("(kc p) -> kc p", p=P)
    pb_r = pb_d.ap().rearrange("(kc p) -> kc p", p=P)
    qb_r = qb_d.ap().rearrange("(oc p) -> oc p", p=P)
    c.nw, c.nb, c.pb, c.qb = [], [], [], []
    for kc in range(CK):
        t = const.tile([P, 1], F32, tag=f"nw{kc}", name=f"nw{kc}")
        nc.sync.dma_start(t, nw_r[kc][:, None])
        c.nw.append(t)
        t = const.tile([P, 1], F32, tag=f"nb{kc}", name=f"nb{kc}")
        nc.sync.dma_start(t, nb_r[kc][:, None])
        c.nb.append(t)
        t = const.tile([P, 1], F32, tag=f"pb{kc}", name=f"pb{kc}")
        nc.sync.dma_start(t, pb_r[kc][:, None])
        c.pb.append(t)
    for oc in range(8):
        t = const.tile([P, 1], F32, tag=f"qb{oc}", name=f"qb{oc}")
        nc.sync.dma_start(t, qb_r[oc][:, None])
        c.qb.append(t)
    c.eps_t = const.tile([NG, 1], F32, tag="eps_t")
    nc.vector.memset(c.eps_t, EPS)
    c.ones_col = const.tile([P, NH], F32, tag="ones_col")
    nc.vector.memset(c.ones_col, 1.0)
    # v bias broadcast across partitions (it indexes the free dim of v tiles)
    c.vb = const.tile([P, 512], F32, tag="vb")
    nc.gpsimd.dma_start(
        c.vb[:, None, :], qb_d.ap()[1024:1536][None, :].partition_broadcast(P))
    return c


def _emit(nc, tc, pools, x_d, out_d, nw_d, nb_d, qw_d, qb_d, pw_d, pb_d):
    const, stage, xp, hp_, qkp, vp, ep, attp, op_, sm, csp, ps, ps2 = pools
    c = _consts(nc, const, nw_d, nb_d, qb_d, pb_d)

    x_r = x_d.ap().rearrange("b (kc p) h w -> b kc p (h w)", p=P)
    o_r = out_d.ap().rearrange("b (kc p) h w -> b kc p (h w)", p=P)

    S = [Ctx() for _ in range(BPC)]

    def emit_gn_stats(s):
        st_ = S[s]
        st_.x, st_.stat2 = [], []
        for kc in range(CK):
            xt = xp.tile([P, L], F32, tag=f"x{kc}", name=f"x{kc}_{s}")
            nc.sync.dma_start(xt, x_r[s, kc])
            st_.x.append(xt)
            bst = sm.tile([P, 2, 6], F32, tag="bst", name="bst")
            nc.vector.bn_stats(out=bst[:, 0, :], in_=xt[:, 0:512])
            nc.vector.bn_stats(out=bst[:, 1, :], in_=xt[:, 512:1024])
            mv = sm.tile([P, 2], F32, tag="mv", name="mv")
            nc.vector.bn_aggr(out=mv, in_=bst)
            st2 = sm.tile([P, 2], F32, tag="st2", name="st2")
            nc.vector.tensor_copy(out=st2[:, 0:1], in_=mv[:, 0:1])
            nc.vector.tensor_tensor(st2[:, 1:2], mv[:, 0:1], mv[:, 0:1], OP.mult)
            nc.vector.tensor_tensor(st2[:, 1:2], st2[:, 1:2], mv[:, 1:2], OP.add)
            st_.stat2.append(st2)

    def emit_gn_apply(s):
        st_ = S[s]
        gps = ps2.tile([P, 512], F32, tag="p2", name="gn_ps")
        for kc in range(CK):
            nc.tensor.matmul(gps[0:NG, 0:2], c.gmask[kc], st_.stat2[kc],
                             start=(kc == 0), stop=(kc == CK - 1))
        gst = sm.tile([NG, 2], F32, tag="gst", name="gst")
        gsb = sm.tile([NG, 2], F32, tag="gsb", name="gsb")
        gtmp = sm.tile([NG, 1], F32, tag="gtmp", name="gtmp")
        nc.vector.tensor_copy(out=gsb, in_=gps[0:NG, 0:2])
        nc.vector.tensor_tensor(gtmp, gsb[:, 0:1], gsb[:, 0:1], OP.mult)
        nc.vector.tensor_tensor(gtmp, gsb[:, 1:2], gtmp, OP.subtract)  # var
        nc.scalar.activation(gtmp, gtmp, AF.Ln, bias=c.eps_t)
        nc.scalar.activation(gst[:, 1:2], gtmp, AF.Exp, scale=-0.5)    # rstd
        nc.vector.tensor_copy(out=gst[:, 0:1], in_=gsb[:, 0:1])        # gmean
        chps = ps2.tile([P, 512], F32, tag="p2", name="gn_ps2")
        for kc in range(CK):
            nc.tensor.matmul(chps[:, kc * 2: kc * 2 + 2],
                             c.bmask[:, kc * P:(kc + 1) * P], gst,
                             start=True, stop=True)
        st_.h = []
        for kc in range(CK):
            Acol = sm.tile([P, 1], F32, tag="Acol", name="Acol")
            Bcol = sm.tile([P, 1], F32, tag="Bcol", name="Bcol")
            nc.vector.tensor_tensor(Acol, chps[:, kc * 2 + 1: kc * 2 + 2],
                                    c.nw[kc], OP.mult)
            nc.vector.tensor_tensor(Bcol, chps[:, kc * 2: kc * 2 + 1], Acol, OP.mult)
            nc.vector.tensor_tensor(Bcol, c.nb[kc], Bcol, OP.subtract)
            ht = hp_.tile([P, L], F32R, tag=f"h{kc}", name=f"h{kc}_{s}")
            nc.vector.tensor_scalar(ht, st_.x[kc], Acol, Bcol, op0=OP.mult, op1=OP.add)
            st_.h.append(ht)
        st_.qkT = [None] * 8
        st_.v = [None] * LK
        st_.att = [None] * CK

    qw_r4 = qw_d.ap().rearrange("(oc p) ch -> oc p ch", p=P)
    pw_r4 = pw_d.ap().rearrange("(oc p) ch -> oc p ch", p=P)
    c.wT = [const.tile([P, 3 * C], F32R, tag=f"wT{kc}", name=f"wT{kc}")
            for kc in range(CK)]
    c.pT = [const.tile([P, C], F32R, tag=f"pT{kc}", name=f"pT{kc}")
            for kc in range(CK)]

    def emit_tr_unit(oc):
        src_r = qw_r4[oc] if oc < 12 else pw_r4[oc - 12]
        dstT = c.wT if oc < 12 else c.pT
        col = (oc if oc < 12 else oc - 12) * P
        ws = stage.tile([P, C], F32, tag="wstage", name="wstage")
        nc.sync.dma_start(ws, src_r)
        pt = ps2.tile([P, 512], F32, tag="p2", name="tr_ps")
        for kc in range(CK):
            nc.tensor.transpose(pt[:, kc * P:(kc + 1) * P],
                                ws[:, kc * P:(kc + 1) * P], c.ident)
        for kc in range(CK):
            nc.any.tensor_copy(out=dstT[kc][:, col:col + P],
                               in_=pt[:, kc * P:(kc + 1) * P])

    def emit_qkv_unit(s, oc, li):
        st_ = S[s]
        if st_.qkT[oc] is None:
            st_.qkT[oc] = qkp.tile([P, L], F32R, tag=f"qk{oc}", name=f"qk{oc}_{s}")
        dst = st_.qkT[oc]
        pt = ps2.tile([P, 512], F32, tag="p2", name="qkv_ps")
        for kc in range(CK):
            nc.tensor.matmul(pt,
                             c.wT[kc][:, oc * P:(oc + 1) * P],
                             st_.h[kc][:, li * 512:(li + 1) * 512],
                             start=(kc == 0), stop=(kc == CK - 1))
        nc.vector.tensor_scalar(dst[:, li * 512:(li + 1) * 512],
                                pt, c.qb[oc], None, op0=OP.add)

    def emit_qkv_qk(s, hp):
        for oc in (hp, 4 + hp):
            for li in range(2):
                emit_qkv_unit(s, oc, li)

    def emit_v(s, lcs):
        st_ = S[s]
        for lc in lcs:
            pt = ps2.tile([P, 512], F32, tag="p2", name="v_ps")
            for kc in range(CK):
                nc.tensor.matmul(pt,
                                 st_.h[kc][:, lc * P:(lc + 1) * P],
                                 c.wT[kc][:, 1024:1536],
                                 start=(kc == 0), stop=(kc == CK - 1))
            vt = vp.tile([P, NH, HD + 1], F32R, tag=f"v{lc}", name=f"v{lc}_{s}")
            nc.vector.tensor_copy(out=vt[:, :, HD:HD + 1], in_=c.ones_col[:, :, None])
            nc.vector.tensor_tensor(
                vt[:, :, 0:HD],
                pt.rearrange("p (h d) -> p h d", d=HD),
                c.vb.rearrange("p (h d) -> p h d", d=HD),
                OP.add)
            st_.v[lc] = vt

    fill_q = []
    pending = Ctx()
    pending.norm = None

    def pop_fill():
        if fill_q:
            fill_q.pop(0)()

    def make_norm2(s, hp, rsum):
        st_ = S[s]

        def norm2():
            for li in range(2):
                rb2 = ps2.tile([P, 512], F32, tag="p2", name="rb2_ps")
                nc.tensor.matmul(rb2, c.sel2, rsum[:, li * 512:(li + 1) * 512],
                                 start=True, stop=True)
                nc.vector.tensor_tensor(
                    st_.att[hp][:, li * 512:(li + 1) * 512],
                    st_.att[hp][:, li * 512:(li + 1) * 512], rb2, OP.mult)
        return norm2

    def emit_pair(s, hp):
        st_ = S[s]
        kT, qT = st_.qkT[4 + hp], st_.qkT[hp]
        st_.att[hp] = attp.tile([P, L], F32R, tag=f"att{hp}", name=f"att{hp}_{s}")
        csum = csp.tile([2, L], F32, tag="csum", name=f"csum_{s}_{hp}")

        def s_mms(ic, jc):
            stile = ps.tile([P, 1024], F32, tag="s", name=f"s_{hp}_{ic}_{jc}")
            for h2 in range(2):
                nc.tensor.matmul(
                    stile[:, h2 * 512:(h2 + 1) * 512],
                    kT[h2 * HD:(h2 + 1) * HD, jc * P:(jc + 1) * P],
                    qT[h2 * HD:(h2 + 1) * HD, ic * 512:(ic + 1) * 512],
                    start=True, stop=True)
            return stile

        for ic in range(2):
            av = ps.tile([P, 1024], F32, tag="s", name=f"av_{hp}_{ic}")
            stile = s_mms(ic, 0)
            for jc in range(LK):
                e_t = ep.tile([P, 1024], F32R, tag="e", name="e_t")
                nc.scalar.activation(e_t, stile, AF.Exp, scale=SCALE)
                # emit next S ahead of this AV so the PE stream runs one step
                # ahead of ScalarE; then soak the PE with one filler unit
                if jc + 1 < LK:
                    stile = s_mms(ic, jc + 1)
                pop_fill()
                for h2 in range(2):
                    nc.tensor.matmul(
                        av[0:HD + 1, h2 * 512:(h2 + 1) * 512],
                        st_.v[jc][:, 2 * hp + h2, :],
                        e_t[:, h2 * 512:(h2 + 1) * 512],
                        start=(jc == 0), stop=(jc == LK - 1))
            for h2 in range(2):
                nc.vector.tensor_copy(
                    out=st_.att[hp][h2 * HD:(h2 + 1) * HD, ic * 512:(ic + 1) * 512],
                    in_=av[0:HD, h2 * 512:(h2 + 1) * 512])
                cstage = sm.tile([1, 512], F32, tag="cstage", name="cstage")
                nc.vector.tensor_copy(
                    out=cstage, in_=av[HD:HD + 1, h2 * 512:(h2 + 1) * 512])
                nc.sync.dma_start(csum[h2:h2 + 1, ic * 512:(ic + 1) * 512], cstage)
        # reciprocal inline: DVE-only, never blocks the PE stream; the PE-side
        # broadcast+multiply is queued and pops ~a pair later when rsum is long
        # since ready
        rsum = csp.tile([2, L], F32R, tag="rsum", name=f"rsum_{s}_{hp}")
        with nc.allow_low_precision(reason="f32r rounding"):
            nc.vector.reciprocal(rsum, csum)
        fill_q.insert(min(len(fill_q), 8), make_norm2(s, hp, rsum))

    def emit_proj_unit(s, oc, li):
        st_ = S[s]
        pt = ps2.tile([P, 512], F32, tag="p2", name="proj_ps")
        for kc in range(CK):
            nc.tensor.matmul(pt,
                             c.pT[kc][:, oc * P:(oc + 1) * P],
                             st_.att[kc][:, li * 512:(li + 1) * 512],
                             start=(kc == 0), stop=(kc == CK - 1))
        xres = op_.tile([P, 512], F32, tag="xres", name="xres")
        nc.sync.dma_start(xres, x_r[s, oc][:, li * 512:(li + 1) * 512])
        ot = op_.tile([P, 512], F32, tag="ot", name="ot")
        nc.vector.tensor_scalar(ot, pt, c.pb[oc], None, op0=OP.add)
        nc.vector.tensor_tensor(ot, ot, xres, OP.add)
        nc.sync.dma_start(o_r[s, oc][:, li * 512:(li + 1) * 512], ot)

    def emit_proj(s):
        for oc in range(CK):
            for li in range(2):
                emit_proj_unit(s, oc, li)

    # ---------------- schedule ----------------
    emit_gn_stats(0)          # x DMA + DVE stats start immediately
    for oc in (0, 4, 8, 9, 10, 11):   # only the transposes pair(0,0) needs
        emit_tr_unit(oc)
    emit_gn_apply(0)
    emit_qkv_qk(0, 0)         # pair(0,0) q/k: its DVE epilogues gate the
    emit_v(0, [0, 1, 2])      # first S-matmuls, so they go before gn(1)
    emit_gn_stats(1)          # sample 1 groupnorm still in the head: its
    emit_gn_apply(1)          # Ln/Exp must not sit between attention exps

    # everything else becomes filler units popped one per attention step; the
    # queue order encodes the just-in-time deadlines (v(0,lc) pops ~3 steps
    # before the AV that consumes it)
    for lc in range(3, LK):
        fill_q.append(lambda lc=lc: emit_v(0, [lc]))
    for oc_t, oc_a, oc_b in ((1, 1, 5), (2, 2, 6), (3, 3, 7)):
        fill_q.append(lambda oc=oc_t: emit_tr_unit(oc))
        fill_q.append(lambda oc=oc_t: emit_tr_unit(oc + 4))
        for li in range(2):
            fill_q.append(lambda oc=oc_a, li=li: emit_qkv_unit(0, oc, li))
        for li in range(2):
            fill_q.append(lambda oc=oc_b, li=li: emit_qkv_unit(0, oc, li))
    for oc in (12, 13, 14, 15):       # proj weights, needed from pair(1,0)
        fill_q.append(lambda oc=oc: emit_tr_unit(oc))
    for oc in (0, 4, 1, 5, 2, 6, 3, 7):
        for li in range(2):
            fill_q.append(lambda oc=oc, li=li: emit_qkv_unit(1, oc, li))
    for lc in range(LK):
        fill_q.append(lambda lc=lc: emit_v(1, [lc]))

    for hp in range(4):
        emit_pair(0, hp)
    while fill_q:
        pop_fill()

    for oc in range(CK):
        for li in range(2):
            fill_q.append(lambda oc=oc, li=li: emit_proj_unit(0, oc, li))
    for hp in range(4):
        emit_pair(1, hp)
    while fill_q:
        pop_fill()
    emit_proj(1)


def _build():
    if "nc" in _NC_CACHE:
        return _NC_CACHE["nc"]
    nc = bacc.Bacc("TRN2", target_bir_lowering=False, debug=False)
    x_d = nc.dram_tensor("x", (BPC, C, H, W), F32, kind="ExternalInput")
    nw_d = nc.dram_tensor("norm_w", (C,), F32, kind="ExternalInput")
    nb_d = nc.dram_tensor("norm_b", (C,), F32, kind="ExternalInput")
    qw_d = nc.dram_tensor("qkv_w", (3 * C, C), F32, kind="ExternalInput")
    qb_d = nc.dram_tensor("qkv_b", (3 * C,), F32, kind="ExternalInput")
    pw_d = nc.dram_tensor("proj_w", (C, C), F32, kind="ExternalInput")
    pb_d = nc.dram_tensor("proj_b", (C,), F32, kind="ExternalInput")
    out_d = nc.dram_tensor("out", (BPC, C, H, W), F32, kind="ExternalOutput")
    with tile.TileContext(nc) as tc:
        with (
            tc.tile_pool(name="const", bufs=1) as const,
            tc.tile_pool(name="stage", bufs=2) as stage,
            tc.tile_pool(name="xp", bufs=1) as xp,
            tc.tile_pool(name="hp", bufs=1) as hp_,
            tc.tile_pool(name="qkp", bufs=1) as qkp,
            tc.tile_pool(name="vp", bufs=2) as vp,
            tc.tile_pool(name="ep", bufs=2) as ep,
            tc.tile_pool(name="attp", bufs=2) as attp,
            tc.tile_pool(name="op", bufs=2) as op_,
            tc.tile_pool(name="sm", bufs=1) as sm,
            tc.tile_pool(name="csp", bufs=2) as csp,
            tc.tile_pool(name="ps", bufs=3, space="PSUM") as ps,
            tc.tile_pool(name="ps2", bufs=2, space="PSUM") as ps2,
        ):
            pools = (const, stage, xp, hp_, qkp, vp, ep, attp, op_, sm, csp, ps, ps2)
            _emit(nc, tc, pools, x_d, out_d, nw_d, nb_d, qw_d, qb_d, pw_d, pb_d)
    nc.compile()
    _NC_CACHE["nc"] = nc
    return nc


def kernel(x, norm_w, norm_b, qkv_w, qkv_b, proj_w, proj_b):
    x = np.ascontiguousarray(x, dtype=np.float32)
    args = {
        "norm_w": np.ascontiguousarray(norm_w, np.float32),
        "norm_b": np.ascontiguousarray(norm_b, np.float32),
        "qkv_w": np.ascontiguousarray(qkv_w, np.float32),
        "qkv_b": np.ascontiguousarray(qkv_b, np.float32),
        "proj_w": np.ascontiguousarray(proj_w, np.float32),
        "proj_b": np.ascontiguousarray(proj_b, np.float32),
    }
    nc = _build()
    in_maps = [dict(args, x=x[i * BPC:(i + 1) * BPC]) for i in range(N_CORES)]
    res = run_bass_kernel_spmd(nc, in_maps, core_ids=list(range(N_CORES)))
    return np.concatenate([r["out"] for r in res.results], axis=0)



# revision 10
# speedup vs baseline: 1.4038x; 1.4038x over previous
"""AttentionBlock (GroupNorm + 8-head self-attention + proj + residual) on 8 trn2 cores.

Sharding: data-parallel over batch B=16 -> 2 samples per core. No collectives.

Per-sample dataflow (C=512 channels, L=1024 pixels, 8 heads x 64 dims):
  - x (C, L) lives as 4 SBUF tiles (128, 1024), channels on partitions.
  - GroupNorm: per-channel mean/var via bn_stats over L; 16-channel group
    aggregation + broadcast-back via tiny mask matmuls on the PE.
  - QKV: q^T,k^T computed as (channels, L) tiles; v computed directly in
    (L, channels) orientation (lhsT = h) so AV needs no transposes.
  - Attention per head pair, split by i-halves so PSUM double-buffers:
    S^T = k^T.T @ q^T chunks (row-packed head pairs share the PE array, K=64
    each); exp on ScalarE with the 1/8 scale fused, PSUM (128,1024) in one
    instruction; AV uses v' = [v | ones] (M=65) so the softmax denominator
    rides along as PSUM row 64. Denominators collect per pair into a (2, L)
    tile; reciprocal + a K=2 selector matmul broadcasts them back to channel
    rows for one normalization multiply per chunk, right after each pair.
  - proj + bias + residual, write out.
  - Cross-sample software pipeline: sample s+1's groupnorm/QKV/V fill the PE
    while ScalarE works through sample s's exps; sample s's proj fills the
    head of sample s+1's attention.

Big matmuls run in float32r (full-rate PE mode, ~1e-4 rel err); producers of
their operands write float32r-rounded outputs as walrus requires.
"""

import numpy as np

import concourse.bass as bass
import concourse.mybir as mybir
import concourse.tile as tile
from concourse import bacc
from concourse.bass_utils import run_bass_kernel_spmd
from concourse.masks import make_identity

F32 = mybir.dt.float32
F32R = mybir.dt.float32r
BF16 = mybir.dt.bfloat16
AF = mybir.ActivationFunctionType
OP = mybir.AluOpType

B, C, H, W = 16, 512, 32, 32
L = H * W
NH, HD = 8, 64
NG, GS = 32, 16
EPS = 1e-5
N_CORES = 8
BPC = B // N_CORES  # samples per core
P = 128
CK = C // P   # 4 channel chunks
LK = L // P   # 8 pixel chunks
SCALE = HD ** -0.5

_NC_CACHE = {}


class Ctx:
    pass


def _consts(nc, const, nw_d, nb_d, qb_d, pb_d):
    c = Ctx()
    c.ident = const.tile([P, P], F32, tag="ident")
    make_identity(nc, c.ident)

    # gmask[kc][ch, g] = 1/16 iff global_channel // 16 == g   (128, 32)
    c.gmask = []
    for kc in range(CK):
        gm = const.tile([P, NG], F32, tag=f"gmask{kc}", name=f"gmask{kc}")
        nc.gpsimd.memset(gm, 1.0 / GS)
        nc.gpsimd.affine_select(
            out=gm, in_=gm, compare_op=OP.is_ge, fill=0.0,
            base=P * kc, channel_multiplier=1, pattern=[[-GS, NG]])
        nc.gpsimd.affine_select(
            out=gm, in_=gm, compare_op=OP.is_ge, fill=0.0,
            base=(GS - 1) - P * kc, channel_multiplier=-1, pattern=[[GS, NG]])
        c.gmask.append(gm)

    # sel2[h2, ch] = 1 iff ch // 64 == h2  (2, 128), f32r for full-rate matmul
    sel2s = const.tile([2, P], F32, tag="sel2s")
    nc.gpsimd.memset(sel2s, 1.0)
    nc.gpsimd.affine_select(
        out=sel2s, in_=sel2s, compare_op=OP.is_ge, fill=0.0,
        base=0, channel_multiplier=-HD, pattern=[[1, P]])
    nc.gpsimd.affine_select(
        out=sel2s, in_=sel2s, compare_op=OP.is_ge, fill=0.0,
        base=HD - 1, channel_multiplier=HD, pattern=[[-1, P]])
    c.sel2 = const.tile([2, P], BF16, tag="sel2")
    nc.vector.tensor_copy(out=c.sel2, in_=sel2s)

    # bmask[g, ch] = 1 iff ch // 16 == g  (32, 512)
    c.bmask = const.tile([NG, C], F32, tag="bmask")
    nc.gpsimd.memset(c.bmask, 1.0)
    nc.gpsimd.affine_select(
        out=c.bmask, in_=c.bmask, compare_op=OP.is_ge, fill=0.0,
        base=0, channel_multiplier=-GS, pattern=[[1, C]])
    nc.gpsimd.affine_select(
        out=c.bmask, in_=c.bmask, compare_op=OP.is_ge, fill=0.0,
        base=GS - 1, channel_multiplier=GS, pattern=[[-1, C]])

    nw_r = nw_d.ap().rearrange("(kc p) -> kc p", p=P)
    nb_r = nb_d.ap().rearrange("(kc p) -> kc p", p=P)
    pb_r = pb_d.ap().rearrange("(kc p) -> kc p", p=P)
    qb_r = qb_d.ap().rearrange("(oc p) -> oc p", p=P)
    c.nw, c.nb, c.pb, c.qb = [], [], [], []
    for kc in range(CK):
        t = const.tile([P, 1], F32, tag=f"nw{kc}", name=f"nw{kc}")
        nc.sync.dma_start(t, nw_r[kc][:, None])
        c.nw.append(t)
        t = const.tile([P, 1], F32, tag=f"nb{kc}", name=f"nb{kc}")
        nc.sync.dma_start(t, nb_r[kc][:, None])
        c.nb.append(t)
        t = const.tile([P, 1], F32, tag=f"pb{kc}", name=f"pb{kc}")
        nc.sync.dma_start(t, pb_r[kc][:, None])
        c.pb.append(t)
    for oc in range(8):
        t = const.tile([P, 1], F32, tag=f"qb{oc}", name=f"qb{oc}")
        nc.sync.dma_start(t, qb_r[oc][:, None])
        c.qb.append(t)
    c.eps_t = const.tile([NG, 1], F32, tag="eps_t")
    nc.vector.memset(c.eps_t, EPS)
    c.ones_col = const.tile([P, NH], F32, tag="ones_col")
    nc.vector.memset(c.ones_col, 1.0)
    # v bias broadcast across partitions (it indexes the free dim of v tiles)
    c.vb = const.tile([P, 512], F32, tag="vb")
    nc.gpsimd.dma_start(
        c.vb[:, None, :], qb_d.ap()[1024:1536][None, :].partition_broadcast(P))
    return c


def _emit(nc, tc, pools, x_d, out_d, nw_d, nb_d, qw_d, qb_d, pw_d, pb_d):
    const, stage, xp, hp_, qkp, vp, ep, attp, op_, sm, csp, ps, ps2 = pools
    c = _consts(nc, const, nw_d, nb_d, qb_d, pb_d)

    x_r = x_d.ap().rearrange("b (kc p) h w -> b kc p (h w)", p=P)
    o_r = out_d.ap().rearrange("b (kc p) h w -> b kc p (h w)", p=P)

    S = [Ctx() for _ in range(BPC)]

    def emit_gn_stats(s):
        st_ = S[s]
        st_.x, st_.stat2 = [], []
        for kc in range(CK):
            xt = xp.tile([P, L], F32, tag=f"x{kc}", name=f"x{kc}_{s}")
            nc.sync.dma_start(xt, x_r[s, kc])
            st_.x.append(xt)
            bst = sm.tile([P, 2, 6], F32, tag="bst", name="bst")
            nc.vector.bn_stats(out=bst[:, 0, :], in_=xt[:, 0:512])
            nc.vector.bn_stats(out=bst[:, 1, :], in_=xt[:, 512:1024])
            mv = sm.tile([P, 2], F32, tag="mv", name="mv")
            nc.vector.bn_aggr(out=mv, in_=bst)
            st2 = sm.tile([P, 2], F32, tag="st2", name="st2")
            nc.vector.tensor_copy(out=st2[:, 0:1], in_=mv[:, 0:1])
            nc.vector.tensor_tensor(st2[:, 1:2], mv[:, 0:1], mv[:, 0:1], OP.mult)
            nc.vector.tensor_tensor(st2[:, 1:2], st2[:, 1:2], mv[:, 1:2], OP.add)
            st_.stat2.append(st2)

    def emit_gn_apply(s):
        st_ = S[s]
        gps = ps2.tile([P, 512], F32, tag="p2", name="gn_ps")
        for kc in range(CK):
            nc.tensor.matmul(gps[0:NG, 0:2], c.gmask[kc], st_.stat2[kc],
                             start=(kc == 0), stop=(kc == CK - 1))
        gst = sm.tile([NG, 2], F32, tag="gst", name="gst")
        gsb = sm.tile([NG, 2], F32, tag="gsb", name="gsb")
        gtmp = sm.tile([NG, 1], F32, tag="gtmp", name="gtmp")
        nc.vector.tensor_copy(out=gsb, in_=gps[0:NG, 0:2])
        nc.vector.tensor_tensor(gtmp, gsb[:, 0:1], gsb[:, 0:1], OP.mult)
        nc.vector.tensor_tensor(gtmp, gsb[:, 1:2], gtmp, OP.subtract)  # var
        nc.scalar.activation(gtmp, gtmp, AF.Ln, bias=c.eps_t)
        nc.scalar.activation(gst[:, 1:2], gtmp, AF.Exp, scale=-0.5)    # rstd
        nc.vector.tensor_copy(out=gst[:, 0:1], in_=gsb[:, 0:1])        # gmean
        chps = ps2.tile([P, 512], F32, tag="p2", name="gn_ps2")
        for kc in range(CK):
            nc.tensor.matmul(chps[:, kc * 2: kc * 2 + 2],
                             c.bmask[:, kc * P:(kc + 1) * P], gst,
                             start=True, stop=True)
        st_.h = []
        for kc in range(CK):
            Acol = sm.tile([P, 1], F32, tag="Acol", name="Acol")
            Bcol = sm.tile([P, 1], F32, tag="Bcol", name="Bcol")
            nc.vector.tensor_tensor(Acol, chps[:, kc * 2 + 1: kc * 2 + 2],
                                    c.nw[kc], OP.mult)
            nc.vector.tensor_tensor(Bcol, chps[:, kc * 2: kc * 2 + 1], Acol, OP.mult)
            nc.vector.tensor_tensor(Bcol, c.nb[kc], Bcol, OP.subtract)
            ht = hp_.tile([P, L], BF16, tag=f"h{kc}", name=f"h{kc}_{s}")
            nc.vector.tensor_scalar(ht, st_.x[kc], Acol, Bcol, op0=OP.mult, op1=OP.add)
            st_.h.append(ht)
        st_.qkT = [None] * 8
        st_.v = [None] * LK
        st_.att = [None] * CK

    qw_r4 = qw_d.ap().rearrange("(oc p) ch -> oc p ch", p=P)
    pw_r4 = pw_d.ap().rearrange("(oc p) ch -> oc p ch", p=P)
    c.wT = [const.tile([P, 3 * C], BF16, tag=f"wT{kc}", name=f"wT{kc}")
            for kc in range(CK)]
    c.pT = [const.tile([P, C], BF16, tag=f"pT{kc}", name=f"pT{kc}")
            for kc in range(CK)]

    def emit_tr_unit(oc):
        src_r = qw_r4[oc] if oc < 12 else pw_r4[oc - 12]
        dstT = c.wT if oc < 12 else c.pT
        col = (oc if oc < 12 else oc - 12) * P
        ws = stage.tile([P, C], F32, tag="wstage", name="wstage")
        nc.sync.dma_start(ws, src_r)
        pt = ps2.tile([P, 512], F32, tag="p2", name="tr_ps")
        for kc in range(CK):
            nc.tensor.transpose(pt[:, kc * P:(kc + 1) * P],
                                ws[:, kc * P:(kc + 1) * P], c.ident)
        for kc in range(CK):
            nc.any.tensor_copy(out=dstT[kc][:, col:col + P],
                               in_=pt[:, kc * P:(kc + 1) * P])

    def emit_qkv_unit(s, oc, li):
        st_ = S[s]
        if st_.qkT[oc] is None:
            st_.qkT[oc] = qkp.tile([P, L], BF16, tag=f"qk{oc}", name=f"qk{oc}_{s}")
        dst = st_.qkT[oc]
        pt = ps2.tile([P, 512], F32, tag="p2", name="qkv_ps")
        for kc in range(CK):
            nc.tensor.matmul(pt,
                             c.wT[kc][:, oc * P:(oc + 1) * P],
                             st_.h[kc][:, li * 512:(li + 1) * 512],
                             start=(kc == 0), stop=(kc == CK - 1))
        nc.vector.tensor_scalar(dst[:, li * 512:(li + 1) * 512],
                                pt, c.qb[oc], None, op0=OP.add)

    def emit_qkv_qk(s, hp):
        for oc in (hp, 4 + hp):
            for li in range(2):
                emit_qkv_unit(s, oc, li)

    def emit_v(s, lcs):
        st_ = S[s]
        for lc in lcs:
            pt = ps2.tile([P, 512], F32, tag="p2", name="v_ps")
            for kc in range(CK):
                nc.tensor.matmul(pt,
                                 st_.h[kc][:, lc * P:(lc + 1) * P],
                                 c.wT[kc][:, 1024:1536],
                                 start=(kc == 0), stop=(kc == CK - 1))
            vt = vp.tile([P, NH, HD + 1], BF16, tag=f"v{lc}", name=f"v{lc}_{s}")
            nc.vector.tensor_copy(out=vt[:, :, HD:HD + 1], in_=c.ones_col[:, :, None])
            nc.vector.tensor_tensor(
                vt[:, :, 0:HD],
                pt.rearrange("p (h d) -> p h d", d=HD),
                c.vb.rearrange("p (h d) -> p h d", d=HD),
                OP.add)
            st_.v[lc] = vt

    fill_q = []
    pending = Ctx()
    pending.norm = None

    def pop_fill():
        if fill_q:
            fill_q.pop(0)()

    def make_norm2(s, hp, rsum):
        st_ = S[s]

        def norm2():
            for li in range(2):
                rb2 = ps2.tile([P, 512], F32, tag="p2", name="rb2_ps")
                nc.tensor.matmul(rb2, c.sel2, rsum[:, li * 512:(li + 1) * 512],
                                 start=True, stop=True)
                nc.vector.tensor_tensor(
                    st_.att[hp][:, li * 512:(li + 1) * 512],
                    st_.att[hp][:, li * 512:(li + 1) * 512], rb2, OP.mult)
        return norm2

    def emit_pair(s, hp):
        st_ = S[s]
        kT, qT = st_.qkT[4 + hp], st_.qkT[hp]
        st_.att[hp] = attp.tile([P, L], BF16, tag=f"att{hp}", name=f"att{hp}_{s}")
        csum = csp.tile([2, L], F32, tag="csum", name=f"csum_{s}_{hp}")

        def s_mms(ic, jc):
            stile = ps.tile([P, 1024], F32, tag="s", name=f"s_{hp}_{ic}_{jc}")
            for h2 in range(2):
                nc.tensor.matmul(
                    stile[:, h2 * 512:(h2 + 1) * 512],
                    kT[h2 * HD:(h2 + 1) * HD, jc * P:(jc + 1) * P],
                    qT[h2 * HD:(h2 + 1) * HD, ic * 512:(ic + 1) * 512],
                    start=True, stop=True)
            return stile

        for ic in range(2):
            av = ps.tile([P, 1024], F32, tag="s", name=f"av_{hp}_{ic}")
            stile = s_mms(ic, 0)
            for jc in range(LK):
                e_t = ep.tile([P, 1024], BF16, tag="e", name="e_t")
                nc.scalar.activation(e_t, stile, AF.Exp, scale=SCALE)
                # emit next S ahead of this AV so the PE stream runs one step
                # ahead of ScalarE; then soak the PE with one filler unit
                if jc + 1 < LK:
                    stile = s_mms(ic, jc + 1)
                pop_fill()
                for h2 in range(2):
                    nc.tensor.matmul(
                        av[0:HD + 1, h2 * 512:(h2 + 1) * 512],
                        st_.v[jc][:, 2 * hp + h2, :],
                        e_t[:, h2 * 512:(h2 + 1) * 512],
                        start=(jc == 0), stop=(jc == LK - 1))
            for h2 in range(2):
                nc.vector.tensor_copy(
                    out=st_.att[hp][h2 * HD:(h2 + 1) * HD, ic * 512:(ic + 1) * 512],
                    in_=av[0:HD, h2 * 512:(h2 + 1) * 512])
                cstage = sm.tile([1, 512], F32, tag="cstage", name="cstage")
                nc.vector.tensor_copy(
                    out=cstage, in_=av[HD:HD + 1, h2 * 512:(h2 + 1) * 512])
                nc.sync.dma_start(csum[h2:h2 + 1, ic * 512:(ic + 1) * 512], cstage)
        # reciprocal inline: DVE-only, never blocks the PE stream; the PE-side
        # broadcast+multiply is queued and pops ~a pair later when rsum is long
        # since ready
        rtmp = csp.tile([2, L], F32, tag="rtmp", name=f"rtmp_{s}_{hp}")
        nc.vector.reciprocal_approx_fast(out=rtmp, in_=csum)
        rsum = csp.tile([2, L], BF16, tag="rsum", name=f"rsum_{s}_{hp}")
        with nc.allow_low_precision(reason="bf16 rounding"):
            nc.vector.tensor_copy(out=rsum, in_=rtmp)
        fill_q.insert(min(len(fill_q), 8), make_norm2(s, hp, rsum))

    def emit_proj_unit(s, oc, li):
        st_ = S[s]
        pt = ps2.tile([P, 512], F32, tag="p2", name="proj_ps")
        for kc in range(CK):
            nc.tensor.matmul(pt,
                             c.pT[kc][:, oc * P:(oc + 1) * P],
                             st_.att[kc][:, li * 512:(li + 1) * 512],
                             start=(kc == 0), stop=(kc == CK - 1))
        xres = op_.tile([P, 512], F32, tag="xres", name="xres")
        nc.sync.dma_start(xres, x_r[s, oc][:, li * 512:(li + 1) * 512])
        ot = op_.tile([P, 512], F32, tag="ot", name="ot")
        nc.vector.tensor_scalar(ot, pt, c.pb[oc], None, op0=OP.add)
        nc.vector.tensor_tensor(ot, ot, xres, OP.add)
        nc.sync.dma_start(o_r[s, oc][:, li * 512:(li + 1) * 512], ot)

    def emit_proj(s):
        for oc in range(CK):
            for li in range(2):
                emit_proj_unit(s, oc, li)

    # ---------------- schedule ----------------
    emit_gn_stats(0)          # x DMA + DVE stats start immediately
    for oc in (0, 4, 8, 9, 10, 11):   # only the transposes pair(0,0) needs
        emit_tr_unit(oc)
    emit_gn_apply(0)
    emit_qkv_qk(0, 0)         # pair(0,0) q/k: its DVE epilogues gate the
    emit_v(0, [0, 1, 2])      # first S-matmuls, so they go before gn(1)
    emit_gn_stats(1)          # sample 1 groupnorm still in the head: its
    emit_gn_apply(1)          # Ln/Exp must not sit between attention exps

    # everything else becomes filler units popped one per attention step; the
    # queue order encodes the just-in-time deadlines (v(0,lc) pops ~3 steps
    # before the AV that consumes it)
    for lc in range(3, LK):
        fill_q.append(lambda lc=lc: emit_v(0, [lc]))
    for oc_t, oc_a, oc_b in ((1, 1, 5), (2, 2, 6), (3, 3, 7)):
        fill_q.append(lambda oc=oc_t: emit_tr_unit(oc))
        fill_q.append(lambda oc=oc_t: emit_tr_unit(oc + 4))
        for li in range(2):
            fill_q.append(lambda oc=oc_a, li=li: emit_qkv_unit(0, oc, li))
        for li in range(2):
            fill_q.append(lambda oc=oc_b, li=li: emit_qkv_unit(0, oc, li))
    for oc in (12, 13, 14, 15):       # proj weights, needed from pair(1,0)
        fill_q.append(lambda oc=oc: emit_tr_unit(oc))
    for oc in (0, 4, 1, 5, 2, 6, 3, 7):
        for li in range(2):
            fill_q.append(lambda oc=oc, li=li: emit_qkv_unit(1, oc, li))
    for lc in range(LK):
        fill_q.append(lambda lc=lc: emit_v(1, [lc]))

    for hp in range(4):
        emit_pair(0, hp)
    while fill_q:
        pop_fill()

    for oc in range(CK):
        for li in range(2):
            fill_q.append(lambda oc=oc, li=li: emit_proj_unit(0, oc, li))
    for hp in range(4):
        emit_pair(1, hp)
    while fill_q:
        pop_fill()
    emit_proj(1)


def _build():
    if "nc" in _NC_CACHE:
        return _NC_CACHE["nc"]
    nc = bacc.Bacc("TRN2", target_bir_lowering=False, debug=False)
    x_d = nc.dram_tensor("x", (BPC, C, H, W), F32, kind="ExternalInput")
    nw_d = nc.dram_tensor("norm_w", (C,), F32, kind="ExternalInput")
    nb_d = nc.dram_tensor("norm_b", (C,), F32, kind="ExternalInput")
    qw_d = nc.dram_tensor("qkv_w", (3 * C, C), F32, kind="ExternalInput")
    qb_d = nc.dram_tensor("qkv_b", (3 * C,), F32, kind="ExternalInput")
    pw_d = nc.dram_tensor("proj_w", (C, C), F32, kind="ExternalInput")
    pb_d = nc.dram_tensor("proj_b", (C,), F32, kind="ExternalInput")
    out_d = nc.dram_tensor("out", (BPC, C, H, W), F32, kind="ExternalOutput")
    with tile.TileContext(nc) as tc:
        with (
            tc.tile_pool(name="const", bufs=1) as const,
            tc.tile_pool(name="stage", bufs=2) as stage,
            tc.tile_pool(name="xp", bufs=1) as xp,
            tc.tile_pool(name="hp", bufs=1) as hp_,
            tc.tile_pool(name="qkp", bufs=1) as qkp,
            tc.tile_pool(name="vp", bufs=2) as vp,
            tc.tile_pool(name="ep", bufs=2) as ep,
            tc.tile_pool(name="attp", bufs=2) as attp,
            tc.tile_pool(name="op", bufs=2) as op_,
            tc.tile_pool(name="sm", bufs=1) as sm,
            tc.tile_pool(name="csp", bufs=2) as csp,
            tc.tile_pool(name="ps", bufs=3, space="PSUM") as ps,
            tc.tile_pool(name="ps2", bufs=2, space="PSUM") as ps2,
        ):
            pools = (const, stage, xp, hp_, qkp, vp, ep, attp, op_, sm, csp, ps, ps2)
            _emit(nc, tc, pools, x_d, out_d, nw_d, nb_d, qw_d, qb_d, pw_d, pb_d)
    nc.compile()
    _NC_CACHE["nc"] = nc
    return nc


def kernel(x, norm_w, norm_b, qkv_w, qkv_b, proj_w, proj_b):
    x = np.ascontiguousarray(x, dtype=np.float32)
    args = {
        "norm_w": np.ascontiguousarray(norm_w, np.float32),
        "norm_b": np.ascontiguousarray(norm_b, np.float32),
        "qkv_w": np.ascontiguousarray(qkv_w, np.float32),
        "qkv_b": np.ascontiguousarray(qkv_b, np.float32),
        "proj_w": np.ascontiguousarray(proj_w, np.float32),
        "proj_b": np.ascontiguousarray(proj_b, np.float32),
    }
    nc = _build()
    in_maps = [dict(args, x=x[i * BPC:(i + 1) * BPC]) for i in range(N_CORES)]
    res = run_bass_kernel_spmd(nc, in_maps, core_ids=list(range(N_CORES)))
    return np.concatenate([r["out"] for r in res.results], axis=0)

